# revision 13
# baseline (speedup 1.0000x reference)
"""GAT/GCN message-passing layer on 8 Trainium2 NeuronCores.

Math: the reference computes, per query node i,
    e[i,j]   = f_src[i] + f_dst[j]           (GAT additive attention, masked by Ahat>0)
    attn     = softmax_j(e masked)
    out      = relu(attn @ h_prime)
The f_src[i] term is constant along the softmax axis, so it cancels:
    attn[i,j] = Ahat[i,j]*exp(f_dst[j]) / sum_k Ahat[i,k]*exp(f_dst[k])
With g = exp(f_dst) the whole layer is one GEMM over the adjacency:
    out = relu( (Ahat @ [g*h_prime | g])[:, :256] / (Ahat @ [g*h_prime | g])[:, 256:] )
where h_prime = node_feats @ w and f_dst = node_feats @ (w @ w_a @ a[2:4]).

Sharding: 1D row partition of query nodes.  Each of the 8 cores owns 1024 rows
of Ahat and computes its 1024x256 slice of the output.  The small tensors
(node_feats^T, [w | u]) are replicated; each core recomputes the
B = [g*h_prime | g] panel locally, which is cheaper than a collective.

The adjacency is binary, so fp8e4m3 is LOSSLESS for it: Ahat ships as fp8
(8.4 MB/core, half the bf16 bytes) pre-packed on the host into the exact SBUF
tile layout (fully-contiguous 8KB descriptor runs) and lives pinned in SBUF.
The PE matmuls mix fp8 stationary (adjacency) with bf16 moving (B panel); the
array runs at bf16 speed either way and fp8 weights load faster via FWL.
params are likewise host-packed partition-major so every DMA chunk is one
descriptor per partition (~0.95us ACT issue instead of ~1.9us).

Engine split: PE streams matmuls nearly gapless (prefix -> main j<32 for banks
2..7 -> catch-up j<32 for banks 0/1 -> j>=32 for all banks, so every PSUM bank
stops at j=63 and the epilogue runs once, in parallel across ACT/DVE).  ACT
computes exp(f_dst) and issues the params DMAs; DVE (otherwise idle) builds
the B panel; SWDGE stores bf16 output (host upcasts, error budget ~4e-3).

A quirk this kernel works around everywhere: this walrus accepts only ONE sync
wait per instruction (the kernel-tail Drain included), so the dataflow is
arranged so every instruction has at most one cross-engine dependency, and a
"funnel" of SP nops observes every semaphore's final value before the drain.
"""

import os
import sys

import ml_dtypes
import numpy as np

sys.path.insert(0, "/opt/trn_rl_repo")

import concourse.bass as bass  # noqa: E402
import concourse.tile as tile  # noqa: E402
from concourse import mybir  # noqa: E402
from concourse.bass_utils import run_bass_kernel_spmd  # noqa: E402
from concourse.tile import add_dep_helper  # noqa: E402

N = 8192
F = 256  # in_features == out_features
FE = F + 1  # h_prime columns + the g column
NCORES = 8
ROWS = N // NCORES  # 1024 output rows per core
P = 128
NJ = N // P  # 64 contraction blocks
NI = ROWS // P  # 8 output-row blocks per core

BF = mybir.dt.bfloat16
FP8 = mybir.dt.float8e4

PJ = 8  # j-blocks per pinned adjacency tile
NPIN = NJ // PJ  # 8 -- everything pinned
JSTART = 32  # banks 0/1 prefix-borrow boundary (caught up mid-stream)

# params flat layout: [wext (2*FE) | nfT (NJ*256)], one descriptor/partition
WCOLS = 2 * FE
PCOLS = WCOLS + NJ * 2 * P
# chunk bounds in flat columns: wext+2j, then ~thirds of the rest
PB = [0, WCOLS + 2 * 256, WCOLS + 22 * 256, WCOLS + 43 * 256, PCOLS]

_CACHE = {}


def _build():
    nc = bass.Bass(
        "TRN2",
        target_bir_lowering=False,
        debug=False,
        enable_asserts=True,
        num_devices=NCORES,
    )
    aT = nc.dram_tensor("aT", [P, NJ * ROWS], FP8, kind="ExternalInput").ap()
    params = nc.dram_tensor("params", [P, PCOLS], BF, kind="ExternalInput").ap()
    out = nc.dram_tensor("out", [ROWS, F], BF, kind="ExternalOutput").ap()

    with tile.TileContext(nc) as tc:
        _body(tc, aT, params, out)
    return nc


def _body(tc, aT, params, out):
    nc = tc.nc
    f32 = mybir.dt.float32
    Exp = mybir.ActivationFunctionType.Exp

    with (
        tc.tile_pool(name="consts", bufs=1) as consts,
        tc.tile_pool(name="opool", bufs=1) as opool,
        tc.tile_pool(name="rpool", bufs=8) as rpool,
        tc.tile_pool(name="psum", bufs=1, space="PSUM") as psum,
    ):
        # ---- loads ---------------------------------------------------------
        # params chunks, ACT-issued upfront so prefix deps collapse onto the
        # Activation semaphore; one descriptor per partition each.
        params_sb = consts.tile([P, PCOLS], BF, tag="params")
        pchunks = []
        for c in range(len(PB) - 1):
            lo, hi = PB[c], PB[c + 1]
            pchunks.append(
                nc.scalar.dma_start(params_sb[:, lo:hi], params[:, lo:hi])
            )

        def wext_sb(kb):
            return params_sb[:, kb * FE : (kb + 1) * FE]

        def nfT_sb(j, kb):
            o = WCOLS + j * 2 * P + kb * P
            return params_sb[:, o : o + P]

        # Adjacency: 8 pinned 1MB fp8 loads, host-packed so each is 128
        # contiguous 8KB runs.  Gate behind the params issues so they don't
        # starve the params chunks on the SDMA fabric.
        pinned = []
        pdmas = []
        for t in range(NPIN):
            pt = consts.tile([P, PJ * ROWS], FP8, tag=f"aTp{t}", name=f"aTp{t}")
            pinned.append(pt)
            pdma = nc.sync.dma_start(
                pt[:], aT[:, t * PJ * ROWS : (t + 1) * PJ * ROWS]
            )
            add_dep_helper(
                pdma.ins, pchunks[0 if t == 0 else -1].ins, reason="params first"
            )
            pdmas.append(pdma)

        def a_lhsT(j, i):
            """SBUF [128, 128] lhsT view of adjacency j-block, i-block i."""
            t = pinned[j // PJ]
            o = j % PJ
            return t[:, o * ROWS + i * P : o * ROWS + (i + 1) * P]

        # ---- PSUM accumulators --------------------------------------------
        acc = [
            psum.tile([P, FE], f32, tag=f"acc{i}", name=f"acc{i}") for i in range(NI)
        ]

        # ---- prefix: B[j] = [g*h_prime | g], all 64 j-blocks ---------------
        # h' matmuls borrow PSUM banks 0/1; those banks' main accumulation is
        # caught up mid-stream (after the prefix) so every bank stops at j=63.
        # ACT computes g = exp(f_dst) (tiny, absorbs the PE stop tick); DVE
        # does the [128,256] g*h' scale into bf16 B and the strided g-column
        # drops, so ACT never paces the pipeline.
        B_all = consts.tile([P, NJ * FE], BF, tag="B")
        btile = [B_all[:, j * FE : (j + 1) * FE] for j in range(NJ)]
        G = consts.tile([P, NJ], f32, tag="G")
        scr = consts.tile([P, 8], f32, tag="scr")
        scr2 = consts.tile([P, 8], f32, tag="scr2")
        prev_act = None
        prev_dve = None
        for j in range(NJ):
            hp = acc[j % 2]
            for kb in range(2):
                nc.tensor.matmul(
                    hp[:],
                    lhsT=nfT_sb(j, kb),
                    rhs=wext_sb(kb),
                    start=(kb == 0),
                    stop=(kb == 1),
                )
            b = btile[j]
            gj = G[:, j : j + 1]
            # PSUM readers are chained by the framework, so hp is read ONLY
            # by DVE: a copy of the f_dst column to SBUF carries the single
            # PE wait; ACT exps from SBUF (single DVE wait); the broadcast
            # tensor_mul re-reads hp with its PE dep elided by same-engine
            # history and carries only the ACT (exp) wait.
            fc = scr[:, j % 8 : j % 8 + 1]
            cp = nc.vector.tensor_copy(fc, hp[:, F : F + 1])
            if prev_dve is not None:
                add_dep_helper(cp.ins, prev_dve.ins, sync=False, reason="dve order")
            ex = nc.scalar.activation(gj, fc, Exp)
            if prev_act is not None:
                add_dep_helper(ex.ins, prev_act.ins, sync=False, reason="act order")
            prev_act = ex
            # second absorber: dead read of g takes the ACT wait onto DVE, so
            # the mul's remaining sync wait is just the (unavoidable) PSUM
            # reader-chain tick on cp
            ab = nc.vector.tensor_copy(scr2[:, j % 8 : j % 8 + 1], gj)
            add_dep_helper(ab.ins, cp.ins, sync=False, reason="dve order")
            mu = nc.vector.tensor_mul(
                b[:, 0:F], hp[:, 0:F], gj.broadcast_to([P, F])
            )
            add_dep_helper(mu.ins, ab.ins, sync=False, reason="dve order")
            prev_dve = mu
            if j % 8 == 7:
                # strided copy drops this 8-group's g column into B (DVE, so
                # the main matmuls' B dependency stays single-engine)
                c0 = j - 7
                prev_dve = nc.vector.tensor_copy(
                    B_all[:, c0 * FE + F : (j + 1) * FE : FE], G[:, c0 : j + 1]
                )

        # ---- main stream ---------------------------------------------------
        # phase A: j<32 for banks 2..7 (B and pins arrive in j order)
        # phase B: catch-up j<32 for banks 0/1 (prefix released their banks)
        # phase C: j>=32 for all banks; every bank stops at j=63
        last_mm = None
        bank_stop = [None] * NI
        for j in range(JSTART):
            for i in range(2, NI):
                last_mm = nc.tensor.matmul(
                    acc[i][:],
                    lhsT=a_lhsT(j, i),
                    rhs=btile[j][:],
                    start=(j == 0),
                    stop=False,
                )
        for j in range(JSTART):
            for i in range(2):
                last_mm = nc.tensor.matmul(
                    acc[i][:],
                    lhsT=a_lhsT(j, i),
                    rhs=btile[j][:],
                    start=(j == 0),
                    stop=False,
                )
        for j in range(JSTART, NJ):
            for i in range(NI):
                mm = nc.tensor.matmul(
                    acc[i][:],
                    lhsT=a_lhsT(j, i),
                    rhs=btile[j][:],
                    start=False,
                    stop=(j == NJ - 1),
                )
                if j == NJ - 1:
                    bank_stop[i] = mm
                last_mm = mm

        # ---- epilogue: out[i] = relu(acc[i][:, :F] / acc[i][:, F]) ---------
        # ACT copies denominators into SBUF in bank order (the chain's final
        # observed PE tick is the global last matmul), DVE takes reciprocals,
        # then four banks relu on ACT (fused relu(acc*recip)) and four on DVE
        # in parallel; SWDGE stores bf16.
        otile = opool.tile([P, NI * F], BF, tag="o")
        stores = []
        denom = rpool.tile([P, NI], f32, tag="denom")
        denom_last = None
        # banks 2..7 first (fresh PE waits, ascending), then 1, 0: banks 0/1
        # carry a DVE reader-chain wait from the prefix, and their PE waits
        # are elided by the earlier copies -- one wait per copy either way.
        for i in [2, 3, 4, 5, 6, 7, 1, 0]:
            dc = nc.scalar.copy(denom[:, i : i + 1], acc[i][:, F : F + 1])
            if denom_last is not None:
                add_dep_helper(dc.ins, denom_last.ins, sync=False, reason="act order")
            denom_last = dc
        recip = rpool.tile([P, NI], f32, tag="recip")
        rec = nc.vector.reciprocal(recip[:], denom[:])
        add_dep_helper(rec.ins, prev_dve.ins, sync=False, reason="dve order")
        # sacrificial reads: absorb recip's DVE tick for the ACT relus, and
        # the recip RAW for the DVE muls (same-engine RAW is a sync wait)
        sac = rpool.tile([P, NI], f32, tag="sac")
        sa = nc.scalar.copy(sac[:], recip[:])
        add_dep_helper(sa.ins, denom_last.ins, sync=False, reason="act order")
        sacd = rpool.tile([P, NI], f32, tag="sacd")
        sd = nc.vector.tensor_copy(sacd[:], recip[:])
        add_dep_helper(sd.ins, rec.ins, sync=False, reason="dve order")
        last_relu = sa
        last_dve = sd
        for i in range(NI):
            o = otile[:, i * F : (i + 1) * F]
            if i < 4:
                rl = nc.scalar.activation(
                    o,
                    acc[i][:, 0:F],
                    mybir.ActivationFunctionType.Relu,
                    scale=recip[:, i : i + 1],
                )
                add_dep_helper(rl.ins, last_relu.ins, sync=False, reason="act order")
                last_relu = rl
            else:
                mm_ = nc.vector.tensor_mul(
                    o, acc[i][:, 0:F], recip[:, i : i + 1].broadcast_to([P, F])
                )
                add_dep_helper(mm_.ins, last_dve.ins, sync=False, reason="dve order")
                last_dve = nc.vector.tensor_scalar_max(o, o, 0.0)
            stores.append(nc.gpsimd.dma_start(out[i * P : (i + 1) * P, :], o))

        # Funnel every proc's final tick into SP via single-wait nops so the
        # kernel-tail drain (which otherwise aggregates many sem waits, far
        # over walrus's 1-wait cap) has nothing left to wait on.  DMAs fan
        # out over several HW-DGE queues (each queue sem needs its own
        # observer), so DMA deps get two nops each; surplus nops lower to
        # wait-free no-ops.
        deps = []
        for d in [*pdmas, *pchunks]:
            deps += [d, d]
        deps += [*stores, *stores[-2:], last_mm, denom_last, last_relu, last_dve]
        for dep in deps:
            nop = nc.sync.nop(nofuse=True, hint="tail_funnel")
            add_dep_helper(nop.ins, dep.ins, reason="tail funnel")


def _prep_inputs(node_feats, Ahat, w, w_a, a):
    node_feats = np.asarray(node_feats, dtype=np.float32)
    Ahat = np.asarray(Ahat, dtype=np.float32)
    w = np.asarray(w, dtype=np.float32)
    w_a = np.asarray(w_a, dtype=np.float32)
    a = np.asarray(a, dtype=np.float32)

    u = w @ (w_a @ a[2:4])  # [256, 1]
    wext = np.concatenate([w, u], axis=1)  # [256, 257]
    # flat params, partition-major: [wext (kb,FE) | nfT (j, kb, c)]
    wext_f = wext.reshape(2, P, FE).transpose(1, 0, 2).reshape(P, 2 * FE)
    nfT = node_feats.T  # [256, 8192]
    nfT_f = (
        nfT.reshape(2, P, NJ, P).transpose(1, 2, 0, 3).reshape(P, NJ * 2 * P)
    )
    params = np.ascontiguousarray(
        np.concatenate([wext_f, nfT_f], axis=1)
    ).astype("bfloat16")

    in_maps = []
    for c in range(NCORES):
        # fp8 adjacency slice, packed into the exact SBUF layout:
        # aT_flat[p, jb*ROWS + c] = A[row0+c, jb*128 + p]
        aT_c = Ahat[c * ROWS : (c + 1) * ROWS, :].T.astype(ml_dtypes.float8_e4m3)
        aT_c = np.ascontiguousarray(
            aT_c.reshape(NJ, P, ROWS).transpose(1, 0, 2).reshape(P, NJ * ROWS)
        )
        in_maps.append({"aT": aT_c, "params": params})
    return in_maps


def _run(inputs, trace=False, **kwargs):
    if "nc" not in _CACHE:
        _CACHE["nc"] = _build()
    nc = _CACHE["nc"]
    in_maps = _prep_inputs(**inputs)
    res = run_bass_kernel_spmd(
        nc, in_maps, core_ids=list(range(NCORES)), trace=trace, **kwargs
    )
    full = np.concatenate(
        [res.results[c]["out"].astype(np.float32) for c in range(NCORES)], axis=0
    )
    return full, res


def kernel(**inputs) -> np.ndarray:
    out, _ = _run(inputs, trace=False)
    return out
